# revision 35
# baseline (speedup 1.0000x reference)
"""Deformable conv (torchvision v1, stride=1 pad=1 K=3) on 8 TRN2 NeuronCores.

Sharding: core i handles sample b=i//2, output-row half i%2 (48 of 96 rows).
Weights replicated; no cross-core communication.

Per-core pipeline (v9 — group-pipelined):
  The 36 128-px units are processed in 3 groups of 12 (16 output rows per
  group).  Per group: A. offset conv (3x3, 256->18) via PE matmuls on a
  per-group xp tile, PE-transposed to pixel-on-partition offs [128,12,18];
  B. sampling math -> bilinear corner weights w4 + gather index, SWDGE-
  wrapped via strided DMAs + replication tree into xw [128,12,9,8] int16;
  C-F per unit: dma_gather (4 SWDGE queues, round-robin) from DRAM xt4
  (row q = 2KB = 4 bilinear corners x 256ch bf16) -> gt [128,9,1024],
  ACT-expanded corner weights, DVE packed-2x multiply + y/x folds ->
  val [128, 18*128] bf16; per block: PE transposes (2 ct / PSUM bank)
  software-pipelined with the 18 accumulating output matmuls; bias via
  ACT, DMA out.  Group g+1's A/B overlaps group g's C-F.
"""

import sys

import numpy as np

if "/opt/trn_rl_repo" not in sys.path:
    sys.path.insert(0, "/opt/trn_rl_repo")

import ml_dtypes  # noqa: E402

bf16 = ml_dtypes.bfloat16

B, C, H, W, O = 4, 256, 96, 96, 256
K, KK = 3, 9
HW = H * W
P = HW // 2                     # 4608 pixels per core
NCHUNK = P // 128               # 36 128-px units
NBLK = P // 512                 # 9 512-px blocks
NGRP = 3                        # groups of 3 blocks (16 rows) each
GU = NCHUNK // NGRP             # 12 units per group
ROWS = 48
CT = 2 * KK                     # 18 contraction tiles
MAGIC = 12582912.0              # 1.5 * 2^23
XTP_ROWS = 97 + HW + 97         # 9410
XT4_ROWS = HW + 97              # 9313 gatherable rows (q in [0, 9312])


# ---------------------------------------------------------------- host prep
def _prep_core_inputs(x, offset_w, offset_b, deform_w, deform_b, core):
    b, half = core // 2, core % 2
    h0 = half * ROWS
    xb = x[b]                                       # [C, H, W] fp32

    xpad = np.zeros((2, 128, 50, 98), dtype=np.float32)
    r_lo, r_hi = h0 - 1, h0 + ROWS + 1
    src_lo, src_hi = max(r_lo, 0), min(r_hi, H)
    d_lo = src_lo - r_lo
    xpad[:, :, d_lo : d_lo + (src_hi - src_lo), 1:97] = xb[
        :, src_lo:src_hi, :
    ].reshape(2, 128, src_hi - src_lo, W)
    # xp3[kx]: columns shifted by kx-1, zero-padded; rows 50, cols 96
    xp = np.stack(
        [xpad[:, :, :, kx : kx + 96] for kx in range(3)], axis=0
    ).astype(bf16)

    # 4-corner gather source: row q = [xT[q] | xT[q+1] | xT[q+96] | xT[q+97]]
    # (97-row lead pad: q = clamp(y0,-1,95)*96 + clamp(x0,-1,95) + 97)
    xtp = np.zeros((XTP_ROWS, 256), dtype=bf16)
    xtp[97 : 97 + HW] = xb.reshape(C, HW).T.astype(bf16)
    xt4 = np.concatenate(
        [
            xtp[0:XT4_ROWS],
            xtp[1 : XT4_ROWS + 1],
            xtp[96 : XT4_ROWS + 96],
            xtp[97 : XT4_ROWS + 97],
        ],
        axis=1,
    )                                               # [9313, 1024]

    # partition-major single-DMA layouts
    ow = np.ascontiguousarray(
        offset_w.reshape(18, 2, 128, KK)
        .transpose(2, 1, 3, 0)
        .reshape(128, 2, KK * 18)
    ).astype(bf16)

    dw = np.ascontiguousarray(
        deform_w.reshape(O, 2, 128, KK).transpose(2, 3, 1, 0).reshape(128, CT * O)
    ).astype(bf16)

    ob = np.ascontiguousarray(np.broadcast_to(offset_b.astype(np.float32), (128, 18)))
    db = np.ascontiguousarray(
        np.broadcast_to(deform_b.reshape(2, 128, 1), (2, 128, 1))
        .transpose(1, 0, 2)
        .reshape(128, 2)
        .astype(np.float32)
    )

    p_local = np.arange(P)
    basey = (h0 + p_local // W).astype(np.float32).reshape(NCHUNK, 128).T
    basex = (p_local % W).astype(np.float32).reshape(NCHUNK, 128).T
    ky = (np.arange(KK) // 3 - 1).astype(np.float32)
    kx = (np.arange(KK) % 3 - 1).astype(np.float32)
    yb = np.ascontiguousarray(basey[:, :, None] + ky[None, None, :])  # [128,36,9]
    xb_all = np.ascontiguousarray(basex[:, :, None] + kx[None, None, :])

    identf = np.eye(128, dtype=np.float32)
    identb = np.eye(128, dtype=np.float32).astype(bf16)

    return {
        "xp": np.ascontiguousarray(xp.reshape(3, 2, 128, 50 * 96)),
        "xt4": np.ascontiguousarray(xt4),
        "ow": ow,
        "dw": dw,
        "ob": ob,
        "db": db,
        "yb": yb,
        "xb": xb_all,
        "identf": identf,
        "identb": identb,
    }


# ---------------------------------------------------------------- bass build
def build_nc(repeat=1, n_queues=4, gt_bufs=2, tap8_batch=True, indirect=True):
    import concourse.bass as bass
    from concourse import bacc, tile

    mybir = bass.mybir
    dt = mybir.dt
    Alu = mybir.AluOpType
    Act = mybir.ActivationFunctionType

    nc = bacc.Bacc(num_swdge_queues=n_queues)

    xp_d = nc.declare_dram_parameter("xp", [3, 2, 128, 50 * 96], dt.bfloat16, isOutput=False)
    xt4_d = nc.declare_dram_parameter("xt4", [XT4_ROWS, 1024], dt.bfloat16, isOutput=False)
    ow_d = nc.declare_dram_parameter("ow", [128, 2, KK * 18], dt.bfloat16, isOutput=False)
    dw_d = nc.declare_dram_parameter("dw", [128, CT * O], dt.bfloat16, isOutput=False)
    ob_d = nc.declare_dram_parameter("ob", [128, 18], dt.float32, isOutput=False)
    db_d = nc.declare_dram_parameter("db", [128, 2], dt.float32, isOutput=False)
    yb_d = nc.declare_dram_parameter("yb", [128, NCHUNK, KK], dt.float32, isOutput=False)
    xb_d = nc.declare_dram_parameter("xb", [128, NCHUNK, KK], dt.float32, isOutput=False)
    identf_d = nc.declare_dram_parameter("identf", [128, 128], dt.float32, isOutput=False)
    identb_d = nc.declare_dram_parameter("identb", [128, 128], dt.bfloat16, isOutput=False)
    out_d = nc.declare_dram_parameter("out", [2, 128, P], dt.float32, isOutput=True)

    reg1024 = nc.gpsimd.to_reg(1024)
    reg512 = nc.gpsimd.to_reg(512)
    reg128 = nc.gpsimd.to_reg(128)

    with tile.TileContext(nc) as tc:
        from contextlib import ExitStack

        with ExitStack() as ctx:
            cst = ctx.enter_context(tc.tile_pool(name="cst", bufs=1))
            xpgp = ctx.enter_context(tc.tile_pool(name="xpg", bufs=2))
            offsp = ctx.enter_context(tc.tile_pool(name="offs", bufs=2))
            scrp = ctx.enter_context(tc.tile_pool(name="scr", bufs=1))
            w4p = ctx.enter_context(tc.tile_pool(name="w4", bufs=2))
            qp = ctx.enter_context(tc.tile_pool(name="qp", bufs=2))
            xwp = ctx.enter_context(tc.tile_pool(name="xw", bufs=2))
            sb = ctx.enter_context(tc.tile_pool(name="sb", bufs=2))
            g_pool = ctx.enter_context(tc.tile_pool(name="gth", bufs=gt_bufs))
            wex_pool = ctx.enter_context(tc.tile_pool(name="wex", bufs=2))
            m_pool = ctx.enter_context(tc.tile_pool(name="mt", bufs=1))
            val_pool = ctx.enter_context(tc.tile_pool(name="val", bufs=4))
            vm_pool = ctx.enter_context(tc.tile_pool(name="vm", bufs=4))
            outp = ctx.enter_context(tc.tile_pool(name="outp", bufs=2))
            ps_oc = ctx.enter_context(tc.tile_pool(name="ps_oc", bufs=2, space="PSUM"))
            ps_t = ctx.enter_context(tc.tile_pool(name="ps_t", bufs=2, space="PSUM"))
            ps_e = ctx.enter_context(tc.tile_pool(name="ps_e", bufs=2, space="PSUM"))
            ps_f = ctx.enter_context(tc.tile_pool(name="ps_f", bufs=2, space="PSUM"))

            # ---- persistent constants / weights.  A/B-critical ones go
            # first on sync; bulky late-needed ones (dw, identb) on scalar.
            ow_sb = cst.tile([128, 2, KK * 18], dt.bfloat16)
            nc.sync.dma_start(out=ow_sb[:], in_=ow_d[:])
            identf_sb = cst.tile([128, 128], dt.float32)
            nc.sync.dma_start(out=identf_sb[:], in_=identf_d[:])
            ob_sb = cst.tile([128, 18], dt.float32)
            nc.sync.dma_start(out=ob_sb[:], in_=ob_d[:])
            yb_sb = cst.tile([128, NCHUNK, KK], dt.float32)
            nc.sync.dma_start(out=yb_sb[:], in_=yb_d[:])
            xb_sb = cst.tile([128, NCHUNK, KK], dt.float32)
            nc.sync.dma_start(out=xb_sb[:], in_=xb_d[:])
            dw_sb = cst.tile([128, CT, O], dt.bfloat16)
            nc.scalar.dma_start(out=dw_sb.rearrange("p a b -> p (a b)"), in_=dw_d[:])
            identb_sb = cst.tile([128, 128], dt.bfloat16)
            nc.scalar.dma_start(out=identb_sb[:], in_=identb_d[:])
            db_sb = cst.tile([128, 2], dt.float32)
            nc.scalar.dma_start(out=db_sb[:], in_=db_d[:])

            def stage_AB(gg):
                """Offset conv + sampling math + index wrap for group gg.
                Returns (w4g, xwg) consumed by stage_CF."""
                # --- A: per-group x tile (rows 16gg-1 .. 16gg+16 of xpad)
                xpg = xpgp.tile([128, 3, 2, 18 * 96], dt.bfloat16, tag="xpg")
                off0 = gg * 16 * 96
                for kx in range(3):
                    for t in range(2):
                        eng = nc.sync if (kx + t) % 2 == 0 else nc.scalar
                        eng.dma_start(
                            out=xpg[:, kx, t],
                            in_=xp_d[kx, t, :, off0 : off0 + 18 * 96],
                        )
                offs_g = offsp.tile([128, GU, 18], dt.float32, tag="offs")
                for bl in range(3):
                    ps = ps_oc.tile([18, 512], dt.float32, tag="psoc")
                    n = 0
                    for t in range(2):
                        for k in range(KK):
                            ky, kx = k // 3, k % 3
                            rhs = xpg[
                                :, kx, t,
                                bl * 512 + ky * 96 : bl * 512 + ky * 96 + 512,
                            ]
                            nc.tensor.matmul(
                                ps[:],
                                lhsT=ow_sb[:, t, k * 18 : (k + 1) * 18],
                                rhs=rhs,
                                start=(n == 0),
                                stop=(n == 17),
                            )
                            n += 1
                    oc_sb = sb.tile([18, 512], dt.float32, tag="ocsb")
                    nc.scalar.copy(oc_sb[:], ps[:])
                    for t3 in range(4):
                        pst = ps_t.tile([128, 18], dt.float32, tag="pst")
                        nc.tensor.transpose(
                            pst[:],
                            oc_sb[:, t3 * 128 : (t3 + 1) * 128],
                            identf_sb[:18, :18],
                        )
                        nc.vector.tensor_tensor(
                            offs_g[:, bl * 4 + t3], pst[:], ob_sb[:], Alu.add
                        )

                # --- B: sampling math over [128, GU, 9] fp32
                scr = scrp.tile([128, 9, GU, KK], dt.float32, tag="scr")
                w4g = w4p.tile([128, GU, KK, 4], dt.float32, tag="w4")
                qf_sb = qp.tile([128, GU, KK], dt.float32, tag="qf")
                qi_sb = qp.tile([128, GU, KK], dt.int16, tag="qi")
                xwg = xwp.tile([128, GU, KK, 8], dt.int16, tag="xw")

                py, px = scr[:, 0], scr[:, 1]
                ty, tx = scr[:, 2], scr[:, 3]
                y0, x0 = scr[:, 4], scr[:, 5]
                t1 = scr[:, 6]
                va, vb = scr[:, 7], scr[:, 8]

                dy = offs_g.rearrange("p n (k two) -> p n k two", two=2)[:, :, :, 0]
                dx = offs_g.rearrange("p n (k two) -> p n k two", two=2)[:, :, :, 1]
                ybg = yb_sb[:, gg * GU : (gg + 1) * GU]
                xbg = xb_sb[:, gg * GU : (gg + 1) * GU]
                nc.vector.tensor_tensor(py, dy, ybg, Alu.add)
                nc.vector.tensor_tensor(px, dx, xbg, Alu.add)
                for (pp, tt, zz) in ((py, ty, y0), (px, tx, x0)):
                    nc.vector.tensor_scalar(
                        out=t1, in0=pp, scalar1=0.49999997, scalar2=MAGIC,
                        op0=Alu.subtract, op1=Alu.add,
                    )
                    nc.vector.tensor_scalar(
                        out=zz, in0=t1, scalar1=MAGIC, scalar2=None,
                        op0=Alu.subtract,
                    )
                    nc.vector.tensor_tensor(tt, pp, zz, Alu.subtract)

                # corner weights -> w4[p, u, k, j]; j = (ycorner, xcorner)
                # va = vy0 = (0<=y0<=95)*(1-ty); vb = vy1 = (-1<=y0<=94)*ty
                nc.vector.tensor_scalar(out=t1, in0=y0, scalar1=0.0, scalar2=None, op0=Alu.is_ge)
                nc.vector.tensor_scalar(out=va, in0=y0, scalar1=95.0, scalar2=None, op0=Alu.is_le)
                nc.vector.tensor_tensor(va, va, t1, Alu.mult)
                nc.vector.tensor_scalar(out=t1, in0=ty, scalar1=-1.0, scalar2=1.0, op0=Alu.mult, op1=Alu.add)
                nc.vector.tensor_tensor(va, va, t1, Alu.mult)
                nc.vector.tensor_scalar(out=t1, in0=y0, scalar1=-1.0, scalar2=None, op0=Alu.is_ge)
                nc.vector.tensor_scalar(out=vb, in0=y0, scalar1=94.0, scalar2=None, op0=Alu.is_le)
                nc.vector.tensor_tensor(vb, vb, t1, Alu.mult)
                nc.vector.tensor_tensor(vb, vb, ty, Alu.mult)
                # t1 = wx0, ty(reused) = wx1
                wx0, wx1 = scr[:, 6], scr[:, 2]
                nc.vector.tensor_scalar(out=wx0, in0=x0, scalar1=0.0, scalar2=None, op0=Alu.is_ge)
                nc.vector.tensor_scalar(out=py, in0=x0, scalar1=95.0, scalar2=None, op0=Alu.is_le)
                nc.vector.tensor_tensor(wx0, wx0, py, Alu.mult)
                nc.vector.tensor_scalar(out=py, in0=tx, scalar1=-1.0, scalar2=1.0, op0=Alu.mult, op1=Alu.add)
                nc.vector.tensor_tensor(wx0, wx0, py, Alu.mult)
                nc.vector.tensor_scalar(out=wx1, in0=x0, scalar1=-1.0, scalar2=None, op0=Alu.is_ge)
                nc.vector.tensor_scalar(out=py, in0=x0, scalar1=94.0, scalar2=None, op0=Alu.is_le)
                nc.vector.tensor_tensor(wx1, wx1, py, Alu.mult)
                nc.vector.tensor_tensor(wx1, wx1, tx, Alu.mult)

                nc.vector.tensor_tensor(w4g[:, :, :, 0], va, wx0, Alu.mult)
                nc.vector.tensor_tensor(w4g[:, :, :, 1], va, wx1, Alu.mult)
                nc.vector.tensor_tensor(w4g[:, :, :, 2], vb, wx0, Alu.mult)
                nc.vector.tensor_tensor(w4g[:, :, :, 3], vb, wx1, Alu.mult)

                # gather index: clamp(y0,-1,95)*96 + clamp(x0,-1,95) + 97
                nc.vector.tensor_scalar(out=va, in0=y0, scalar1=-1.0, scalar2=95.0, op0=Alu.max, op1=Alu.min)
                nc.vector.tensor_scalar(out=vb, in0=x0, scalar1=-1.0, scalar2=95.0, op0=Alu.max, op1=Alu.min)
                nc.vector.tensor_scalar(out=vb, in0=vb, scalar1=97.0, scalar2=None, op0=Alu.add)
                nc.vector.scalar_tensor_tensor(
                    out=qf_sb[:], in0=va, scalar=96.0, in1=vb,
                    op0=Alu.mult, op1=Alu.add,
                )
                if indirect:
                    # per-(px, tap) row indices for the indirect DMA gather
                    qi32 = qp.tile([128, GU, KK], dt.int32, tag="qi32")
                    nc.vector.tensor_copy(qi32[:], qf_sb[:])
                    return w4g, qi32, None
                nc.vector.tensor_copy(qi_sb[:], qf_sb[:])
                # SWDGE wrap, big gathers (taps 0-7, per unit):
                #   idx j = k*128 + pp  ->  xwg[j%16, u, k, pp//16]
                # tap-8 gathers are batched per 512-px block (512 idxs):
                #   idx j = ul*128 + pp ->  xw8g[j%16, bl, ul, pp//16]
                xw8g = xwp.tile([128, 3, 4, 8], dt.int16, tag="xw8")
                wrap_engs = [nc.sync, nc.scalar]
                for g8 in range(8):
                    wrap_engs[g8 % 2].dma_start(
                        out=xwg[0:16, :, :, g8],
                        in_=qi_sb[g8 * 16 : (g8 + 1) * 16],
                    )
                    wrap_engs[(g8 + 1) % 2].dma_start(
                        out=xw8g[0:16, :, :, g8].rearrange("p a b -> p (a b)"),
                        in_=qi_sb[g8 * 16 : (g8 + 1) * 16, :, 8],
                    )
                # replication tree 16 -> 128
                nc.sync.dma_start(out=xwg[16:32], in_=xwg[0:16])
                nc.scalar.dma_start(out=xw8g[16:32], in_=xw8g[0:16])
                nc.sync.dma_start(out=xwg[32:64], in_=xwg[0:32])
                nc.scalar.dma_start(out=xw8g[32:64], in_=xw8g[0:32])
                nc.sync.dma_start(out=xwg[64:128], in_=xwg[0:64])
                nc.scalar.dma_start(out=xw8g[64:128], in_=xw8g[0:64])
                return w4g, xwg, xw8g

            def stage_CF(gg, w4g, xwg, xw8g):
                """Gather + combine + transpose + output matmuls, group gg."""
                qi32 = xwg if indirect else None
                for bl in range(3):
                    Bb = gg * 3 + bl
                    pms = [
                        ps_f.tile([128, 512], dt.float32, tag="pmf", name=f"pm{_oh}")
                        for _oh in range(2)
                    ]
                    gt8 = None
                    if tap8_batch and not indirect:
                        # tap-8 for all 4 units of this block: one gather
                        gt8 = g_pool.tile([128, 4, 1024], dt.bfloat16, tag="g8")
                        nc.gpsimd.dma_gather(
                            out_ap=gt8[:],
                            in_ap=xt4_d[:],
                            idxs_ap=xw8g[:, bl].rearrange("p u g -> p (u g)"),
                            num_idxs=512,
                            num_idxs_reg=reg512,
                            elem_size=1024,
                            queue_num=0,
                        )
                    val_u = []
                    for qc in range(4):
                        ul = bl * 4 + qc
                        if indirect:
                            # HW-DGE indirect row gather, one per tap:
                            # gt[p, k, :] = xt4[qi32[p, ul, k], :]
                            gt = g_pool.tile([128, KK, 1024], dt.bfloat16, tag="g")
                            for k in range(KK):
                                nc.gpsimd.indirect_dma_start(
                                    out=gt[:, k],
                                    out_offset=None,
                                    in_=xt4_d[:],
                                    in_offset=bass.IndirectOffsetOnAxis(
                                        ap=qi32[:, ul, k : k + 1], axis=0
                                    ),
                                )
                        else:
                            # taps 0-7 (Q7 idx scratch caps num_idxs at 1024)
                            nt = 8 if tap8_batch else KK
                            gt = g_pool.tile([128, nt, 1024], dt.bfloat16, tag="g")
                            nc.gpsimd.dma_gather(
                                out_ap=gt[:, 0:8],
                                in_ap=xt4_d[:],
                                idxs_ap=xwg[:, ul, 0:8].rearrange("p k g -> p (k g)"),
                                num_idxs=1024,
                                num_idxs_reg=reg1024,
                                elem_size=1024,
                                queue_num=0,
                            )
                            if not tap8_batch:
                                nc.gpsimd.dma_gather(
                                    out_ap=gt[:, 8:9],
                                    in_ap=xt4_d[:],
                                    idxs_ap=xw8g[:, bl, qc],
                                    num_idxs=128,
                                    num_idxs_reg=reg128,
                                    elem_size=1024,
                                    queue_num=0,
                                )
                        # ACT: expand corner weights to c128 granularity
                        wexpc = wex_pool.tile([128, KK, 4, 128], dt.bfloat16, tag="wex")
                        nc.scalar.copy(
                            wexpc[:],
                            w4g[:, ul].unsqueeze(3).broadcast_to((128, KK, 4, 128)),
                        )
                        # DVE: packed-2x multiply + y-fold + x-fold
                        m = m_pool.tile([128, KK, 4, 2, 128], dt.bfloat16, tag="m")
                        for chh in range(2):
                            if tap8_batch and not indirect:
                                nc.vector.tensor_tensor(
                                    m[:, 0:8, :, chh],
                                    gt.rearrange(
                                        "p k (j ch c) -> p k j ch c", j=4, ch=2
                                    )[:, :, :, chh],
                                    wexpc[:, 0:8],
                                    Alu.mult,
                                )
                                nc.vector.tensor_tensor(
                                    m[:, 8:9, :, chh],
                                    gt8.rearrange(
                                        "p u (j ch c) -> p u j ch c", j=4, ch=2
                                    )[:, qc : qc + 1, :, chh],
                                    wexpc[:, 8:9],
                                    Alu.mult,
                                )
                            else:
                                nc.vector.tensor_tensor(
                                    m[:, :, :, chh],
                                    gt.rearrange(
                                        "p k (j ch c) -> p k j ch c", j=4, ch=2
                                    )[:, :, :, chh],
                                    wexpc[:],
                                    Alu.mult,
                                )
                        s = m_pool.tile([128, KK, 2, 2, 128], dt.bfloat16, tag="s")
                        nc.vector.tensor_tensor(
                            s[:], m[:, :, 0:2], m[:, :, 2:4], Alu.add
                        )
                        vu = val_pool.tile([128, CT * 128], dt.bfloat16, tag="val")
                        nc.vector.tensor_tensor(
                            vu.rearrange("p (k ch c) -> p k ch c", k=KK, ch=2),
                            s[:, :, 0],
                            s[:, :, 1],
                            Alu.add,
                        )
                        val_u.append(vu)
                    # PE transposes (2 ct / PSUM bank), software-pipelined
                    # with the accumulating output matmuls.
                    vm_tiles = [None] * (CT // 2)

                    def emit_E(ct2):
                        ptr = ps_e.tile([128, 2, 512], dt.bfloat16, tag="pe")
                        for half in range(2):
                            ct_i = ct2 * 2 + half
                            for qc in range(4):
                                nc.tensor.transpose(
                                    ptr[:, half, qc * 128 : (qc + 1) * 128],
                                    val_u[qc][:, ct_i * 128 : (ct_i + 1) * 128],
                                    identb_sb[:],
                                )
                        vmt = vm_pool.tile([128, 2, 512], dt.bfloat16, tag="vm")
                        nc.scalar.copy(vmt[:], ptr[:])
                        vm_tiles[ct2] = vmt

                    def emit_F(ct2):
                        vmt = vm_tiles[ct2]
                        for half in range(2):
                            ct_i = ct2 * 2 + half
                            for oh in range(2):
                                nc.tensor.matmul(
                                    pms[oh][:],
                                    lhsT=dw_sb[:, ct_i, oh * 128 : (oh + 1) * 128],
                                    rhs=vmt[:, half],
                                    start=(ct_i == 0),
                                    stop=(ct_i == CT - 1),
                                )

                    emit_E(0)
                    for ct2 in range(1, CT // 2):
                        emit_E(ct2)
                        emit_F(ct2 - 1)
                    emit_F(CT // 2 - 1)

                    for oh in range(2):
                        ob_t = outp.tile([128, 512], dt.float32, tag="ot")
                        nc.scalar.activation(
                            out=ob_t[:], in_=pms[oh][:],
                            func=Act.Identity, bias=db_sb[:, oh : oh + 1], scale=1.0,
                        )
                        nc.sync.dma_start(
                            out=out_d[oh, :, Bb * 512 : (Bb + 1) * 512], in_=ob_t[:]
                        )

            for _rep in range(repeat):
                wx0g = stage_AB(0)
                wx1g = stage_AB(1)
                stage_CF(0, *wx0g)
                wx2g = stage_AB(2)
                stage_CF(1, *wx1g)
                stage_CF(2, *wx2g)

    # Post-scheduling queue assignment: Tile pins DMASW sem lane
    # (final_position % 8) to whatever SWDGE queue first updates it, so
    # the queue must be a pure function of the lane.  The scheduler may
    # reorder Pool DMAs relative to emission, so assign queues here from
    # the final instruction order.
    lane = 0
    for f in nc.m.functions:
        for bb in f.blocks:
            for inst in bb.instructions:
                if (
                    inst.engine == mybir.EngineType.Pool
                    and type(inst).__name__ == "InstDMAGatherAnt"
                ):
                    inst.queue_num = (lane % 8) % n_queues
                    lane += 1

    nc.compile()
    return nc


# ------------------------------------------------------------ main entry
_NC_CACHE = {}


def _get_nc():
    if "nc" not in _NC_CACHE:
        _NC_CACHE["nc"] = build_nc()
    return _NC_CACHE["nc"]


def _assemble(results):
    out = np.empty((B, O, H, W), dtype=np.float32)
    for core in range(8):
        b, half = core // 2, core % 2
        o = np.asarray(results[core]["out"]).reshape(O, ROWS, W)
        out[b, :, half * ROWS : (half + 1) * ROWS, :] = o
    return out


def kernel(x, offset_w, offset_b, deform_w, deform_b, **_ignored):
    from concourse.bass_utils import run_bass_kernel_spmd

    x = np.asarray(x, dtype=np.float32)
    offset_w = np.asarray(offset_w, dtype=np.float32)
    offset_b = np.asarray(offset_b, dtype=np.float32)
    deform_w = np.asarray(deform_w, dtype=np.float32)
    deform_b = np.asarray(deform_b, dtype=np.float32)

    nc = _get_nc()
    in_maps = [
        _prep_core_inputs(x, offset_w, offset_b, deform_w, deform_b, core)
        for core in range(8)
    ]
    res = run_bass_kernel_spmd(nc, in_maps, core_ids=list(range(8)))
    return _assemble([res.results[i] for i in range(8)])


# revision 40
# speedup vs baseline: 1.2922x; 1.2922x over previous
"""Deformable conv (torchvision v1, stride=1 pad=1 K=3) on 8 TRN2 NeuronCores.

Sharding: core i handles sample b=i//2, output-row half i%2 (48 of 96 rows).
Weights replicated; no cross-core communication.

Per-core pipeline (v9 — group-pipelined):
  The 36 128-px units are processed in 3 groups of 12 (16 output rows per
  group).  Per group: A. offset conv (3x3, 256->18) via PE matmuls on a
  per-group xp tile, PE-transposed to pixel-on-partition offs [128,12,18];
  B. sampling math -> bilinear corner weights w4 + gather index, SWDGE-
  wrapped via strided DMAs + replication tree into xw [128,12,9,8] int16;
  C-F per unit: dma_gather (4 SWDGE queues, round-robin) from DRAM xt4
  (row q = 2KB = 4 bilinear corners x 256ch bf16) -> gt [128,9,1024],
  ACT-expanded corner weights, DVE packed-2x multiply + y/x folds ->
  val [128, 18*128] bf16; per block: PE transposes (2 ct / PSUM bank)
  software-pipelined with the 18 accumulating output matmuls; bias via
  ACT, DMA out.  Group g+1's A/B overlaps group g's C-F.
"""

import sys

import numpy as np

if "/opt/trn_rl_repo" not in sys.path:
    sys.path.insert(0, "/opt/trn_rl_repo")

import ml_dtypes  # noqa: E402

bf16 = ml_dtypes.bfloat16

B, C, H, W, O = 4, 256, 96, 96, 256
K, KK = 3, 9
HW = H * W
P = HW // 2                     # 4608 pixels per core
NCHUNK = P // 128               # 36 128-px units
NBLK = P // 512                 # 9 512-px blocks
NGRP = 3                        # groups of 3 blocks (16 rows) each
GU = NCHUNK // NGRP             # 12 units per group
ROWS = 48
CT = 2 * KK                     # 18 contraction tiles
MAGIC = 12582912.0              # 1.5 * 2^23
XTP_ROWS = 97 + HW + 97         # 9410
XT4_ROWS = HW + 97              # 9313 gatherable rows (q in [0, 9312])


# ---------------------------------------------------------------- host prep
def _prep_core_inputs(x, offset_w, offset_b, deform_w, deform_b, core):
    b, half = core // 2, core % 2
    h0 = half * ROWS
    xb = x[b]                                       # [C, H, W] fp32

    xpad = np.zeros((2, 128, 50, 98), dtype=np.float32)
    r_lo, r_hi = h0 - 1, h0 + ROWS + 1
    src_lo, src_hi = max(r_lo, 0), min(r_hi, H)
    d_lo = src_lo - r_lo
    xpad[:, :, d_lo : d_lo + (src_hi - src_lo), 1:97] = xb[
        :, src_lo:src_hi, :
    ].reshape(2, 128, src_hi - src_lo, W)
    # xp3[kx]: columns shifted by kx-1, zero-padded; rows 50, cols 96
    xp = np.stack(
        [xpad[:, :, :, kx : kx + 96] for kx in range(3)], axis=0
    ).astype(bf16)

    # 4-corner gather source: row q = [xT[q] | xT[q+1] | xT[q+96] | xT[q+97]]
    # (97-row lead pad: q = clamp(y0,-1,95)*96 + clamp(x0,-1,95) + 97)
    xtp = np.zeros((XTP_ROWS, 256), dtype=bf16)
    xtp[97 : 97 + HW] = xb.reshape(C, HW).T.astype(bf16)
    xt4 = np.concatenate(
        [
            xtp[0:XT4_ROWS],
            xtp[1 : XT4_ROWS + 1],
            xtp[96 : XT4_ROWS + 96],
            xtp[97 : XT4_ROWS + 97],
        ],
        axis=1,
    )                                               # [9313, 1024]

    # partition-major single-DMA layouts
    ow = np.ascontiguousarray(
        offset_w.reshape(18, 2, 128, KK)
        .transpose(2, 1, 3, 0)
        .reshape(128, 2, KK * 18)
    ).astype(bf16)

    dw = np.ascontiguousarray(
        deform_w.reshape(O, 2, 128, KK).transpose(2, 3, 1, 0).reshape(128, CT * O)
    ).astype(bf16)

    ob = np.ascontiguousarray(np.broadcast_to(offset_b.astype(np.float32), (128, 18)))
    db = np.ascontiguousarray(
        np.broadcast_to(deform_b.reshape(2, 128, 1), (2, 128, 1))
        .transpose(1, 0, 2)
        .reshape(128, 2)
        .astype(np.float32)
    )

    p_local = np.arange(P)
    basey = (h0 + p_local // W).astype(np.float32).reshape(NCHUNK, 128).T
    basex = (p_local % W).astype(np.float32).reshape(NCHUNK, 128).T
    ky = (np.arange(KK) // 3 - 1).astype(np.float32)
    kx = (np.arange(KK) % 3 - 1).astype(np.float32)
    yb = np.ascontiguousarray(basey[:, :, None] + ky[None, None, :])  # [128,36,9]
    xb_all = np.ascontiguousarray(basex[:, :, None] + kx[None, None, :])

    identf = np.eye(128, dtype=np.float32)
    identb = np.eye(128, dtype=np.float32).astype(bf16)

    return {
        "xp": np.ascontiguousarray(xp.reshape(3, 2, 128, 50 * 96)),
        "xt4": np.ascontiguousarray(xt4),
        "ow": ow,
        "dw": dw,
        "ob": ob,
        "db": db,
        "yb": yb,
        "xb": xb_all,
        "identf": identf,
        "identb": identb,
    }


# ---------------------------------------------------------------- bass build
def build_nc(repeat=1, n_queues=4, gt_bufs=2, tap8_batch=True, indirect=False):
    import concourse.bass as bass
    from concourse import bacc, tile

    mybir = bass.mybir
    dt = mybir.dt
    Alu = mybir.AluOpType
    Act = mybir.ActivationFunctionType

    nc = bacc.Bacc(num_swdge_queues=n_queues)

    xp_d = nc.declare_dram_parameter("xp", [3, 2, 128, 50 * 96], dt.bfloat16, isOutput=False)
    xt4_d = nc.declare_dram_parameter("xt4", [XT4_ROWS, 1024], dt.bfloat16, isOutput=False)
    ow_d = nc.declare_dram_parameter("ow", [128, 2, KK * 18], dt.bfloat16, isOutput=False)
    dw_d = nc.declare_dram_parameter("dw", [128, CT * O], dt.bfloat16, isOutput=False)
    ob_d = nc.declare_dram_parameter("ob", [128, 18], dt.float32, isOutput=False)
    db_d = nc.declare_dram_parameter("db", [128, 2], dt.float32, isOutput=False)
    yb_d = nc.declare_dram_parameter("yb", [128, NCHUNK, KK], dt.float32, isOutput=False)
    xb_d = nc.declare_dram_parameter("xb", [128, NCHUNK, KK], dt.float32, isOutput=False)
    identf_d = nc.declare_dram_parameter("identf", [128, 128], dt.float32, isOutput=False)
    identb_d = nc.declare_dram_parameter("identb", [128, 128], dt.bfloat16, isOutput=False)
    out_d = nc.declare_dram_parameter("out", [2, 128, P], dt.float32, isOutput=True)

    reg1024 = nc.gpsimd.to_reg(1024)
    reg512 = nc.gpsimd.to_reg(512)
    reg128 = nc.gpsimd.to_reg(128)

    with tile.TileContext(nc) as tc:
        from contextlib import ExitStack

        with ExitStack() as ctx:
            cst = ctx.enter_context(tc.tile_pool(name="cst", bufs=1))
            xpgp = ctx.enter_context(tc.tile_pool(name="xpg", bufs=2))
            offsp = ctx.enter_context(tc.tile_pool(name="offs", bufs=2))
            scrp = ctx.enter_context(tc.tile_pool(name="scr", bufs=1))
            w4p = ctx.enter_context(tc.tile_pool(name="w4", bufs=2))
            qp = ctx.enter_context(tc.tile_pool(name="qp", bufs=2))
            xwp = ctx.enter_context(tc.tile_pool(name="xw", bufs=2))
            sb = ctx.enter_context(tc.tile_pool(name="sb", bufs=2))
            g_pool = ctx.enter_context(tc.tile_pool(name="gth", bufs=gt_bufs))
            wex_pool = ctx.enter_context(tc.tile_pool(name="wex", bufs=2))
            m_pool = ctx.enter_context(tc.tile_pool(name="mt", bufs=1))
            val_pool = ctx.enter_context(tc.tile_pool(name="val", bufs=4))
            vm_pool = ctx.enter_context(tc.tile_pool(name="vm", bufs=4))
            outp = ctx.enter_context(tc.tile_pool(name="outp", bufs=2))
            ps_oc = ctx.enter_context(tc.tile_pool(name="ps_oc", bufs=2, space="PSUM"))
            ps_t = ctx.enter_context(tc.tile_pool(name="ps_t", bufs=2, space="PSUM"))
            ps_e = ctx.enter_context(tc.tile_pool(name="ps_e", bufs=2, space="PSUM"))
            ps_f = ctx.enter_context(tc.tile_pool(name="ps_f", bufs=2, space="PSUM"))

            # ---- persistent constants / weights.  A/B-critical ones go
            # first on sync; bulky late-needed ones (dw, identb) on scalar.
            ow_sb = cst.tile([128, 2, KK * 18], dt.bfloat16)
            nc.sync.dma_start(out=ow_sb[:], in_=ow_d[:])
            identf_sb = cst.tile([128, 128], dt.float32)
            nc.sync.dma_start(out=identf_sb[:], in_=identf_d[:])
            ob_sb = cst.tile([128, 18], dt.float32)
            nc.sync.dma_start(out=ob_sb[:], in_=ob_d[:])
            yb_sb = cst.tile([128, NCHUNK, KK], dt.float32)
            nc.sync.dma_start(out=yb_sb[:], in_=yb_d[:])
            xb_sb = cst.tile([128, NCHUNK, KK], dt.float32)
            nc.sync.dma_start(out=xb_sb[:], in_=xb_d[:])
            dw_sb = cst.tile([128, CT, O], dt.bfloat16)
            nc.scalar.dma_start(out=dw_sb.rearrange("p a b -> p (a b)"), in_=dw_d[:])
            identb_sb = cst.tile([128, 128], dt.bfloat16)
            nc.scalar.dma_start(out=identb_sb[:], in_=identb_d[:])
            db_sb = cst.tile([128, 2], dt.float32)
            nc.scalar.dma_start(out=db_sb[:], in_=db_d[:])

            def load_xpg(gg):
                """Load group gg's x tile (rows 16gg-1 .. 16gg+16 of xpad)."""
                xpg = xpgp.tile([128, 3, 2, 18 * 96], dt.bfloat16, tag="xpg")
                off0 = gg * 16 * 96
                for kx in range(3):
                    for t in range(2):
                        eng = nc.sync if (kx + t) % 2 == 0 else nc.scalar
                        eng.dma_start(
                            out=xpg[:, kx, t],
                            in_=xp_d[kx, t, :, off0 : off0 + 18 * 96],
                        )
                return xpg

            def stage_AB(gg, xpg):
                """Offset conv + sampling math + index wrap for group gg.
                Returns (w4g, xwg) consumed by stage_CF."""
                offs_g = offsp.tile([128, GU, 18], dt.float32, tag="offs")
                for bl in range(3):
                    ps = ps_oc.tile([18, 512], dt.float32, tag="psoc")
                    n = 0
                    for t in range(2):
                        for k in range(KK):
                            ky, kx = k // 3, k % 3
                            rhs = xpg[
                                :, kx, t,
                                bl * 512 + ky * 96 : bl * 512 + ky * 96 + 512,
                            ]
                            nc.tensor.matmul(
                                ps[:],
                                lhsT=ow_sb[:, t, k * 18 : (k + 1) * 18],
                                rhs=rhs,
                                start=(n == 0),
                                stop=(n == 17),
                            )
                            n += 1
                    oc_sb = sb.tile([18, 512], dt.float32, tag="ocsb")
                    nc.scalar.copy(oc_sb[:], ps[:])
                    for t3 in range(4):
                        pst = ps_t.tile([128, 18], dt.float32, tag="pst")
                        nc.tensor.transpose(
                            pst[:],
                            oc_sb[:, t3 * 128 : (t3 + 1) * 128],
                            identf_sb[:18, :18],
                        )
                        nc.vector.tensor_tensor(
                            offs_g[:, bl * 4 + t3], pst[:], ob_sb[:], Alu.add
                        )

                # --- B: sampling math over [128, GU, 9] fp32
                scr = scrp.tile([128, 9, GU, KK], dt.float32, tag="scr")
                w4g = w4p.tile([128, GU, KK, 4], dt.float32, tag="w4")
                qf_sb = qp.tile([128, GU, KK], dt.float32, tag="qf")
                qi_sb = qp.tile([128, GU, KK], dt.int16, tag="qi")
                xwg = xwp.tile([128, GU, KK, 8], dt.int16, tag="xw")

                py, px = scr[:, 0], scr[:, 1]
                ty, tx = scr[:, 2], scr[:, 3]
                y0, x0 = scr[:, 4], scr[:, 5]
                t1 = scr[:, 6]
                va, vb = scr[:, 7], scr[:, 8]

                dy = offs_g.rearrange("p n (k two) -> p n k two", two=2)[:, :, :, 0]
                dx = offs_g.rearrange("p n (k two) -> p n k two", two=2)[:, :, :, 1]
                ybg = yb_sb[:, gg * GU : (gg + 1) * GU]
                xbg = xb_sb[:, gg * GU : (gg + 1) * GU]
                nc.vector.tensor_tensor(py, dy, ybg, Alu.add)
                nc.vector.tensor_tensor(px, dx, xbg, Alu.add)
                for (pp, tt, zz) in ((py, ty, y0), (px, tx, x0)):
                    nc.vector.tensor_scalar(
                        out=t1, in0=pp, scalar1=0.49999997, scalar2=MAGIC,
                        op0=Alu.subtract, op1=Alu.add,
                    )
                    nc.vector.tensor_scalar(
                        out=zz, in0=t1, scalar1=MAGIC, scalar2=None,
                        op0=Alu.subtract,
                    )
                    nc.vector.tensor_tensor(tt, pp, zz, Alu.subtract)

                # corner weights -> w4[p, u, k, j]; j = (ycorner, xcorner)
                # va = vy0 = (0<=y0<=95)*(1-ty); vb = vy1 = (-1<=y0<=94)*ty
                nc.vector.tensor_scalar(out=t1, in0=y0, scalar1=0.0, scalar2=None, op0=Alu.is_ge)
                nc.vector.tensor_scalar(out=va, in0=y0, scalar1=95.0, scalar2=None, op0=Alu.is_le)
                nc.vector.tensor_tensor(va, va, t1, Alu.mult)
                nc.vector.tensor_scalar(out=t1, in0=ty, scalar1=-1.0, scalar2=1.0, op0=Alu.mult, op1=Alu.add)
                nc.vector.tensor_tensor(va, va, t1, Alu.mult)
                nc.vector.tensor_scalar(out=t1, in0=y0, scalar1=-1.0, scalar2=None, op0=Alu.is_ge)
                nc.vector.tensor_scalar(out=vb, in0=y0, scalar1=94.0, scalar2=None, op0=Alu.is_le)
                nc.vector.tensor_tensor(vb, vb, t1, Alu.mult)
                nc.vector.tensor_tensor(vb, vb, ty, Alu.mult)
                # t1 = wx0, ty(reused) = wx1
                wx0, wx1 = scr[:, 6], scr[:, 2]
                nc.vector.tensor_scalar(out=wx0, in0=x0, scalar1=0.0, scalar2=None, op0=Alu.is_ge)
                nc.vector.tensor_scalar(out=py, in0=x0, scalar1=95.0, scalar2=None, op0=Alu.is_le)
                nc.vector.tensor_tensor(wx0, wx0, py, Alu.mult)
                nc.vector.tensor_scalar(out=py, in0=tx, scalar1=-1.0, scalar2=1.0, op0=Alu.mult, op1=Alu.add)
                nc.vector.tensor_tensor(wx0, wx0, py, Alu.mult)
                nc.vector.tensor_scalar(out=wx1, in0=x0, scalar1=-1.0, scalar2=None, op0=Alu.is_ge)
                nc.vector.tensor_scalar(out=py, in0=x0, scalar1=94.0, scalar2=None, op0=Alu.is_le)
                nc.vector.tensor_tensor(wx1, wx1, py, Alu.mult)
                nc.vector.tensor_tensor(wx1, wx1, tx, Alu.mult)

                nc.vector.tensor_tensor(w4g[:, :, :, 0], va, wx0, Alu.mult)
                nc.vector.tensor_tensor(w4g[:, :, :, 1], va, wx1, Alu.mult)
                nc.vector.tensor_tensor(w4g[:, :, :, 2], vb, wx0, Alu.mult)
                nc.vector.tensor_tensor(w4g[:, :, :, 3], vb, wx1, Alu.mult)

                # gather index: clamp(y0,-1,95)*96 + clamp(x0,-1,95) + 97
                nc.vector.tensor_scalar(out=va, in0=y0, scalar1=-1.0, scalar2=95.0, op0=Alu.max, op1=Alu.min)
                nc.vector.tensor_scalar(out=vb, in0=x0, scalar1=-1.0, scalar2=95.0, op0=Alu.max, op1=Alu.min)
                nc.vector.tensor_scalar(out=vb, in0=vb, scalar1=97.0, scalar2=None, op0=Alu.add)
                nc.vector.scalar_tensor_tensor(
                    out=qf_sb[:], in0=va, scalar=96.0, in1=vb,
                    op0=Alu.mult, op1=Alu.add,
                )
                if indirect:
                    # per-(px, tap) row indices for the indirect DMA gather
                    qi32 = qp.tile([128, GU, KK], dt.int32, tag="qi32")
                    nc.vector.tensor_copy(qi32[:], qf_sb[:])
                    return w4g, qi32, None
                nc.vector.tensor_copy(qi_sb[:], qf_sb[:])
                # SWDGE idx wrap, big gathers (taps 0-7, per unit):
                #   idx j = k*128 + pp  ->  xwg[pp%16, u, k, pp//16]
                # tap-8 gathers are batched per 512-px block (512 idxs):
                #   idx j = ul*128 + pp ->  xw8g[pp%16, bl, ul, pp//16]
                # Done as a coarse partition fold (8 contiguous DMAs) into
                # qi_t[g, s, u, k], then one DVE shuffle to wrapped layout —
                # the direct strided-2-byte wrap DMAs measured 2-5.6us each.
                qi_t = qp.tile([16, 8, GU, KK], dt.int16, tag="qit")
                for s in range(8):
                    eng = nc.sync if s % 2 == 0 else nc.scalar
                    eng.dma_start(
                        out=qi_t[:, s], in_=qi_sb[s * 16 : (s + 1) * 16]
                    )
                xw8g = xwp.tile([128, 3, 4, 8], dt.int16, tag="xw8")
                nc.vector.tensor_copy(
                    xwg[0:16], qi_t.rearrange("p s u k -> p u k s")
                )
                nc.vector.tensor_copy(
                    xw8g[0:16].rearrange("p a b s -> p (a b) s"),
                    qi_t[:, :, :, 8].rearrange("p s u -> p u s"),
                )
                # replication tree 16 -> 128
                nc.sync.dma_start(out=xwg[16:32], in_=xwg[0:16])
                nc.scalar.dma_start(out=xw8g[16:32], in_=xw8g[0:16])
                nc.sync.dma_start(out=xwg[32:64], in_=xwg[0:32])
                nc.scalar.dma_start(out=xw8g[32:64], in_=xw8g[0:32])
                nc.sync.dma_start(out=xwg[64:128], in_=xwg[0:64])
                nc.scalar.dma_start(out=xw8g[64:128], in_=xw8g[0:64])
                return w4g, xwg, xw8g

            def stage_CF(gg, w4g, xwg, xw8g):
                """Gather + combine + transpose + output matmuls, group gg."""
                qi32 = xwg if indirect else None
                for bl in range(3):
                    Bb = gg * 3 + bl
                    pms = [
                        ps_f.tile([128, 512], dt.float32, tag="pmf", name=f"pm{_oh}")
                        for _oh in range(2)
                    ]
                    gt8 = None
                    if tap8_batch and not indirect:
                        # tap-8 for all 4 units of this block: one gather
                        gt8 = g_pool.tile([128, 4, 1024], dt.bfloat16, tag="g8")
                        nc.gpsimd.dma_gather(
                            out_ap=gt8[:],
                            in_ap=xt4_d[:],
                            idxs_ap=xw8g[:, bl].rearrange("p u g -> p (u g)"),
                            num_idxs=512,
                            num_idxs_reg=reg512,
                            elem_size=1024,
                            queue_num=0,
                        )
                    val_u = []
                    for qc in range(4):
                        ul = bl * 4 + qc
                        if indirect:
                            # HW-DGE indirect row gather, one per tap:
                            # gt[p, k, :] = xt4[qi32[p, ul, k], :]
                            gt = g_pool.tile([128, KK, 1024], dt.bfloat16, tag="g")
                            for k in range(KK):
                                nc.gpsimd.indirect_dma_start(
                                    out=gt[:, k],
                                    out_offset=None,
                                    in_=xt4_d[:],
                                    in_offset=bass.IndirectOffsetOnAxis(
                                        ap=qi32[:, ul, k : k + 1], axis=0
                                    ),
                                )
                        else:
                            # taps 0-7 (Q7 idx scratch caps num_idxs at 1024)
                            nt = 8 if tap8_batch else KK
                            gt = g_pool.tile([128, nt, 1024], dt.bfloat16, tag="g")
                            nc.gpsimd.dma_gather(
                                out_ap=gt[:, 0:8],
                                in_ap=xt4_d[:],
                                idxs_ap=xwg[:, ul, 0:8].rearrange("p k g -> p (k g)"),
                                num_idxs=1024,
                                num_idxs_reg=reg1024,
                                elem_size=1024,
                                queue_num=0,
                            )
                            if not tap8_batch:
                                nc.gpsimd.dma_gather(
                                    out_ap=gt[:, 8:9],
                                    in_ap=xt4_d[:],
                                    idxs_ap=xw8g[:, bl, qc],
                                    num_idxs=128,
                                    num_idxs_reg=reg128,
                                    elem_size=1024,
                                    queue_num=0,
                                )
                        # ACT: expand corner weights to c128 granularity
                        wexpc = wex_pool.tile([128, KK, 4, 128], dt.bfloat16, tag="wex")
                        nc.scalar.copy(
                            wexpc[:],
                            w4g[:, ul].unsqueeze(3).broadcast_to((128, KK, 4, 128)),
                        )
                        # DVE: packed-2x multiply + y-fold + x-fold
                        m = m_pool.tile([128, KK, 4, 2, 128], dt.bfloat16, tag="m")
                        for chh in range(2):
                            if tap8_batch and not indirect:
                                nc.vector.tensor_tensor(
                                    m[:, 0:8, :, chh],
                                    gt.rearrange(
                                        "p k (j ch c) -> p k j ch c", j=4, ch=2
                                    )[:, :, :, chh],
                                    wexpc[:, 0:8],
                                    Alu.mult,
                                )
                                nc.vector.tensor_tensor(
                                    m[:, 8:9, :, chh],
                                    gt8.rearrange(
                                        "p u (j ch c) -> p u j ch c", j=4, ch=2
                                    )[:, qc : qc + 1, :, chh],
                                    wexpc[:, 8:9],
                                    Alu.mult,
                                )
                            else:
                                nc.vector.tensor_tensor(
                                    m[:, :, :, chh],
                                    gt.rearrange(
                                        "p k (j ch c) -> p k j ch c", j=4, ch=2
                                    )[:, :, :, chh],
                                    wexpc[:],
                                    Alu.mult,
                                )
                        s = m_pool.tile([128, KK, 2, 2, 128], dt.bfloat16, tag="s")
                        nc.vector.tensor_tensor(
                            s[:], m[:, :, 0:2], m[:, :, 2:4], Alu.add
                        )
                        vu = val_pool.tile([128, CT * 128], dt.bfloat16, tag="val")
                        nc.vector.tensor_tensor(
                            vu.rearrange("p (k ch c) -> p k ch c", k=KK, ch=2),
                            s[:, :, 0],
                            s[:, :, 1],
                            Alu.add,
                        )
                        val_u.append(vu)
                    # PE transposes (2 ct / PSUM bank), software-pipelined
                    # with the accumulating output matmuls.
                    vm_tiles = [None] * (CT // 2)

                    def emit_E(ct2):
                        ptr = ps_e.tile([128, 2, 512], dt.bfloat16, tag="pe")
                        for half in range(2):
                            ct_i = ct2 * 2 + half
                            for qc in range(4):
                                nc.tensor.transpose(
                                    ptr[:, half, qc * 128 : (qc + 1) * 128],
                                    val_u[qc][:, ct_i * 128 : (ct_i + 1) * 128],
                                    identb_sb[:],
                                )
                        vmt = vm_pool.tile([128, 2, 512], dt.bfloat16, tag="vm")
                        nc.scalar.copy(vmt[:], ptr[:])
                        vm_tiles[ct2] = vmt

                    def emit_F(ct2):
                        vmt = vm_tiles[ct2]
                        for half in range(2):
                            ct_i = ct2 * 2 + half
                            for oh in range(2):
                                nc.tensor.matmul(
                                    pms[oh][:],
                                    lhsT=dw_sb[:, ct_i, oh * 128 : (oh + 1) * 128],
                                    rhs=vmt[:, half],
                                    start=(ct_i == 0),
                                    stop=(ct_i == CT - 1),
                                )

                    emit_E(0)
                    for ct2 in range(1, CT // 2):
                        emit_E(ct2)
                        emit_F(ct2 - 1)
                    emit_F(CT // 2 - 1)

                    for oh in range(2):
                        ob_t = outp.tile([128, 512], dt.float32, tag="ot")
                        nc.scalar.activation(
                            out=ob_t[:], in_=pms[oh][:],
                            func=Act.Identity, bias=db_sb[:, oh : oh + 1], scale=1.0,
                        )
                        nc.sync.dma_start(
                            out=out_d[oh, :, Bb * 512 : (Bb + 1) * 512], in_=ob_t[:]
                        )

            for _rep in range(repeat):
                xpg_a = load_xpg(0)
                xpg_b = load_xpg(1)
                wx0g = stage_AB(0, xpg_a)
                xpg_c = load_xpg(2)
                wx1g = stage_AB(1, xpg_b)
                stage_CF(0, *wx0g)
                wx2g = stage_AB(2, xpg_c)
                stage_CF(1, *wx1g)
                stage_CF(2, *wx2g)

    # Post-scheduling queue assignment: Tile pins DMASW sem lane
    # (final_position % 8) to whatever SWDGE queue first updates it, so
    # the queue must be a pure function of the lane.  The scheduler may
    # reorder Pool DMAs relative to emission, so assign queues here from
    # the final instruction order.
    lane = 0
    for f in nc.m.functions:
        for bb in f.blocks:
            for inst in bb.instructions:
                if (
                    inst.engine == mybir.EngineType.Pool
                    and type(inst).__name__ == "InstDMAGatherAnt"
                ):
                    inst.queue_num = (lane % 8) % n_queues
                    lane += 1

    nc.compile()
    return nc


# ------------------------------------------------------------ main entry
_NC_CACHE = {}


def _get_nc():
    if "nc" not in _NC_CACHE:
        _NC_CACHE["nc"] = build_nc()
    return _NC_CACHE["nc"]


def _assemble(results):
    out = np.empty((B, O, H, W), dtype=np.float32)
    for core in range(8):
        b, half = core // 2, core % 2
        o = np.asarray(results[core]["out"]).reshape(O, ROWS, W)
        out[b, :, half * ROWS : (half + 1) * ROWS, :] = o
    return out


def kernel(x, offset_w, offset_b, deform_w, deform_b, **_ignored):
    from concourse.bass_utils import run_bass_kernel_spmd

    x = np.asarray(x, dtype=np.float32)
    offset_w = np.asarray(offset_w, dtype=np.float32)
    offset_b = np.asarray(offset_b, dtype=np.float32)
    deform_w = np.asarray(deform_w, dtype=np.float32)
    deform_b = np.asarray(deform_b, dtype=np.float32)

    nc = _get_nc()
    in_maps = [
        _prep_core_inputs(x, offset_w, offset_b, deform_w, deform_b, core)
        for core in range(8)
    ]
    res = run_bass_kernel_spmd(nc, in_maps, core_ids=list(range(8)))
    return _assemble([res.results[i] for i in range(8)])


# revision 74
# speedup vs baseline: 1.4767x; 1.1427x over previous
"""Deformable conv (torchvision v1, stride=1 pad=1 K=3) on 8 TRN2 NeuronCores.

Sharding: core i handles sample b=i//2, output-row half i%2 (48 of 96 rows).
Weights replicated; no cross-core communication.

Per-core pipeline (v9.5 — group-pipelined, hybrid gather):
  The 36 128-px units are processed in 3 groups of 12 (16 output rows per
  group).  Per group: A. offset conv (3x3, 256->18) via PE matmuls on a
  per-group xp tile, PE-transposed to pixel-on-partition offs [128,12,18];
  B. sampling math -> bilinear corner weights w4 + gather row index;
  indices SWDGE-wrapped via a coarse partition fold (8 contiguous DMAs)
  + one DVE shuffle + replication tree into xw [128,12,9,8] int16.
  C-F per unit: taps 0-7 via one 1024-idx SWDGE dma_gather from DRAM xt4
  (row q = 2KB = 4 bilinear corners x 256ch bf16), tap 8 via a HW-DGE
  indirect row gather into the same gt [128,9,1024] tile (gt_bufs=3
  pipelines gather latency); ACT-expanded corner weights, DVE packed-2x
  multiply + y/x folds -> val [128, 18*128] bf16; per block: PE
  transposes (2 ct / PSUM bank) software-pipelined with the 18
  accumulating output matmuls; bias via ACT, DMA out.  Group g+1's A/B
  overlaps group g's C-F; pools rotate across repeat bodies so rep
  boundaries pipeline with ~no stall.

Measured (8 cores SPMD, device-resident repeat-delta, R=50): ~455-480
us/body (ambient-dependent; v8 baseline measured 683 us in the same
window), rel err 5.5e-3 vs fp32 reference (gate 2e-2).
"""

import sys

import numpy as np

if "/opt/trn_rl_repo" not in sys.path:
    sys.path.insert(0, "/opt/trn_rl_repo")

import ml_dtypes  # noqa: E402

bf16 = ml_dtypes.bfloat16

B, C, H, W, O = 4, 256, 96, 96, 256
K, KK = 3, 9
HW = H * W
P = HW // 2                     # 4608 pixels per core
NCHUNK = P // 128               # 36 128-px units
NBLK = P // 512                 # 9 512-px blocks
NGRP = 3                        # groups of 3 blocks (16 rows) each
GU = NCHUNK // NGRP             # 12 units per group
ROWS = 48
CT = 2 * KK                     # 18 contraction tiles
MAGIC = 12582912.0              # 1.5 * 2^23
XTP_ROWS = 97 + HW + 97         # 9410
XT4_ROWS = HW + 97              # 9313 gatherable rows (q in [0, 9312])


# ---------------------------------------------------------------- host prep
def _prep_core_inputs(x, offset_w, offset_b, deform_w, deform_b, core):
    b, half = core // 2, core % 2
    h0 = half * ROWS
    xb = x[b]                                       # [C, H, W] fp32

    xpad = np.zeros((2, 128, 50, 98), dtype=np.float32)
    r_lo, r_hi = h0 - 1, h0 + ROWS + 1
    src_lo, src_hi = max(r_lo, 0), min(r_hi, H)
    d_lo = src_lo - r_lo
    xpad[:, :, d_lo : d_lo + (src_hi - src_lo), 1:97] = xb[
        :, src_lo:src_hi, :
    ].reshape(2, 128, src_hi - src_lo, W)
    # xp3[kx]: columns shifted by kx-1, zero-padded; rows 50, cols 96
    xp = np.stack(
        [xpad[:, :, :, kx : kx + 96] for kx in range(3)], axis=0
    ).astype(bf16)

    # 4-corner gather source: row q = [xT[q] | xT[q+1] | xT[q+96] | xT[q+97]]
    # (97-row lead pad: q = clamp(y0,-1,95)*96 + clamp(x0,-1,95) + 97)
    xtp = np.zeros((XTP_ROWS, 256), dtype=bf16)
    xtp[97 : 97 + HW] = xb.reshape(C, HW).T.astype(bf16)
    xt4 = np.concatenate(
        [
            xtp[0:XT4_ROWS],
            xtp[1 : XT4_ROWS + 1],
            xtp[96 : XT4_ROWS + 96],
            xtp[97 : XT4_ROWS + 97],
        ],
        axis=1,
    )                                               # [9313, 1024]

    # partition-major single-DMA layouts
    ow = np.ascontiguousarray(
        offset_w.reshape(18, 2, 128, KK)
        .transpose(2, 1, 3, 0)
        .reshape(128, 2, KK * 18)
    ).astype(bf16)

    dw = np.ascontiguousarray(
        deform_w.reshape(O, 2, 128, KK).transpose(2, 3, 1, 0).reshape(128, CT * O)
    ).astype(bf16)

    ob = np.ascontiguousarray(np.broadcast_to(offset_b.astype(np.float32), (128, 18)))
    db = np.ascontiguousarray(
        np.broadcast_to(deform_b.reshape(2, 128, 1), (2, 128, 1))
        .transpose(1, 0, 2)
        .reshape(128, 2)
        .astype(np.float32)
    )

    p_local = np.arange(P)
    basey = (h0 + p_local // W).astype(np.float32).reshape(NCHUNK, 128).T
    basex = (p_local % W).astype(np.float32).reshape(NCHUNK, 128).T
    ky = (np.arange(KK) // 3 - 1).astype(np.float32)
    kx = (np.arange(KK) % 3 - 1).astype(np.float32)
    yb = np.ascontiguousarray(basey[:, :, None] + ky[None, None, :])  # [128,36,9]
    xb_all = np.ascontiguousarray(basex[:, :, None] + kx[None, None, :])

    identf = np.eye(128, dtype=np.float32)
    identb = np.eye(128, dtype=np.float32).astype(bf16)

    return {
        "xp": np.ascontiguousarray(xp.reshape(3, 2, 128, 50 * 96)),
        "xt4": np.ascontiguousarray(xt4),
        "ow": ow,
        "dw": dw,
        "ob": ob,
        "db": db,
        "yb": yb,
        "xb": xb_all,
        "identf": identf,
        "identb": identb,
    }


# ---------------------------------------------------------------- bass build
def build_nc(repeat=1, n_queues=4, gt_bufs=3, tap8_batch=True, indirect=False,
             tap8_indirect=True, val_bufs=4, pipe0=False):
    import concourse.bass as bass
    from concourse import bacc, tile

    mybir = bass.mybir
    dt = mybir.dt
    Alu = mybir.AluOpType
    Act = mybir.ActivationFunctionType

    if tap8_indirect:
        # Pool-issued indirect DMAs share the DMASW sem lanes and are
        # modeled as SWDGE queue 0; mixed-queue gathers would conflict.
        n_queues = 1
    nc = bacc.Bacc(num_swdge_queues=n_queues)

    xp_d = nc.declare_dram_parameter("xp", [3, 2, 128, 50 * 96], dt.bfloat16, isOutput=False)
    xt4_d = nc.declare_dram_parameter("xt4", [XT4_ROWS, 1024], dt.bfloat16, isOutput=False)
    ow_d = nc.declare_dram_parameter("ow", [128, 2, KK * 18], dt.bfloat16, isOutput=False)
    dw_d = nc.declare_dram_parameter("dw", [128, CT * O], dt.bfloat16, isOutput=False)
    ob_d = nc.declare_dram_parameter("ob", [128, 18], dt.float32, isOutput=False)
    db_d = nc.declare_dram_parameter("db", [128, 2], dt.float32, isOutput=False)
    yb_d = nc.declare_dram_parameter("yb", [128, NCHUNK, KK], dt.float32, isOutput=False)
    xb_d = nc.declare_dram_parameter("xb", [128, NCHUNK, KK], dt.float32, isOutput=False)
    identf_d = nc.declare_dram_parameter("identf", [128, 128], dt.float32, isOutput=False)
    identb_d = nc.declare_dram_parameter("identb", [128, 128], dt.bfloat16, isOutput=False)
    out_d = nc.declare_dram_parameter("out", [2, 128, P], dt.float32, isOutput=True)

    reg1024 = nc.gpsimd.to_reg(1024)
    reg512 = nc.gpsimd.to_reg(512)
    reg128 = nc.gpsimd.to_reg(128)

    with tile.TileContext(nc) as tc:
        from contextlib import ExitStack

        with ExitStack() as ctx:
            cst = ctx.enter_context(tc.tile_pool(name="cst", bufs=1))
            xpgp = ctx.enter_context(tc.tile_pool(name="xpg", bufs=2))
            offsp = ctx.enter_context(tc.tile_pool(name="offs", bufs=2))
            scrp = ctx.enter_context(tc.tile_pool(name="scr", bufs=1))
            w4p = ctx.enter_context(tc.tile_pool(name="w4", bufs=2))
            qp = ctx.enter_context(tc.tile_pool(name="qp", bufs=2))
            xwp = ctx.enter_context(tc.tile_pool(name="xw", bufs=2))
            sb = ctx.enter_context(tc.tile_pool(name="sb", bufs=2))
            g_pool = ctx.enter_context(tc.tile_pool(name="gth", bufs=gt_bufs))
            wex_pool = ctx.enter_context(tc.tile_pool(name="wex", bufs=2))
            m_pool = ctx.enter_context(tc.tile_pool(name="mt", bufs=1))
            val_pool = ctx.enter_context(tc.tile_pool(name="val", bufs=val_bufs))
            vm_pool = ctx.enter_context(tc.tile_pool(name="vm", bufs=4))
            outp = ctx.enter_context(tc.tile_pool(name="outp", bufs=2))
            ps_oc = ctx.enter_context(tc.tile_pool(name="ps_oc", bufs=2, space="PSUM"))
            ps_t = ctx.enter_context(tc.tile_pool(name="ps_t", bufs=2, space="PSUM"))
            ps_e = ctx.enter_context(tc.tile_pool(name="ps_e", bufs=2, space="PSUM"))
            ps_f = ctx.enter_context(tc.tile_pool(name="ps_f", bufs=2, space="PSUM"))

            # ---- persistent constants / weights.  A/B-critical ones go
            # first on sync; bulky late-needed ones (dw, identb) on scalar.
            ow_sb = cst.tile([128, 2, KK * 18], dt.bfloat16)
            nc.sync.dma_start(out=ow_sb[:], in_=ow_d[:])
            identf_sb = cst.tile([128, 128], dt.float32)
            nc.sync.dma_start(out=identf_sb[:], in_=identf_d[:])
            identb_sb = cst.tile([128, 128], dt.bfloat16)
            nc.sync.dma_start(out=identb_sb[:], in_=identb_d[:])
            # PE p-state warmup: keep the PE busy during input loads so the
            # offset-conv matmuls don't run at the low-power clock.
            wps = ps_e.tile([128, 2, 512], dt.bfloat16, tag="pe", name="warm")
            for _w in range(24):
                nc.tensor.transpose(wps[:, 0, 0:128], identb_sb[:], identb_sb[:])
            ob_sb = cst.tile([128, 18], dt.float32)
            nc.sync.dma_start(out=ob_sb[:], in_=ob_d[:])
            yb_sb = cst.tile([128, NCHUNK, KK], dt.float32)
            nc.sync.dma_start(out=yb_sb[:], in_=yb_d[:])
            xb_sb = cst.tile([128, NCHUNK, KK], dt.float32)
            nc.sync.dma_start(out=xb_sb[:], in_=xb_d[:])
            dw_sb = cst.tile([128, CT, O], dt.bfloat16)
            nc.scalar.dma_start(out=dw_sb.rearrange("p a b -> p (a b)"), in_=dw_d[:])
            db_sb = cst.tile([128, 2], dt.float32)
            nc.scalar.dma_start(out=db_sb[:], in_=db_d[:])

            def load_xpg(gg):
                """Load group gg's x tile (rows 16gg-1 .. 16gg+16 of xpad)."""
                xpg = xpgp.tile([128, 3, 2, 18 * 96], dt.bfloat16, tag="xpg")
                off0 = gg * 16 * 96
                for kx in range(3):
                    for t in range(2):
                        eng = nc.sync if (kx + t) % 2 == 0 else nc.scalar
                        eng.dma_start(
                            out=xpg[:, kx, t],
                            in_=xp_d[kx, t, :, off0 : off0 + 18 * 96],
                        )
                return xpg

            def stage_AB(gg, xpg, pipelined=False):
                """Offset conv + sampling math + index wrap for group gg.
                pipelined=True interleaves B/wrap per 512-px block so the
                first gathers start ~40us earlier (used for group 0)."""
                offs_g = offsp.tile([128, GU, 18], dt.float32, tag="offs")
                scr = scrp.tile([128, 9, GU, KK], dt.float32, tag="scr")
                w4g = w4p.tile([128, GU, KK, 4], dt.float32, tag="w4")
                qf_sb = qp.tile([128, GU, KK], dt.float32, tag="qf")
                qi_sb = qp.tile([128, GU, KK], dt.int16, tag="qi")
                xwg = xwp.tile([128, GU, KK, 8], dt.int16, tag="xw")
                qi32_8 = None
                if tap8_indirect:
                    qi32_8 = qp.tile([128, GU], dt.int32, tag="qi328")

                def emit_A_block(bl):
                    ps = ps_oc.tile([18, 512], dt.float32, tag="psoc")
                    n = 0
                    for t in range(2):
                        for k in range(KK):
                            ky, kx = k // 3, k % 3
                            rhs = xpg[
                                :, kx, t,
                                bl * 512 + ky * 96 : bl * 512 + ky * 96 + 512,
                            ]
                            nc.tensor.matmul(
                                ps[:],
                                lhsT=ow_sb[:, t, k * 18 : (k + 1) * 18],
                                rhs=rhs,
                                start=(n == 0),
                                stop=(n == 17),
                            )
                            n += 1
                    oc_sb = sb.tile([18, 512], dt.float32, tag="ocsb")
                    nc.scalar.copy(oc_sb[:], ps[:])
                    for t3 in range(4):
                        pst = ps_t.tile([128, 18], dt.float32, tag="pst")
                        nc.tensor.transpose(
                            pst[:],
                            oc_sb[:, t3 * 128 : (t3 + 1) * 128],
                            identf_sb[:18, :18],
                        )
                        nc.vector.tensor_tensor(
                            offs_g[:, bl * 4 + t3], pst[:], ob_sb[:], Alu.add
                        )

                def emit_B_range(u0, u1):
                    """Sampling math -> w4/qf for units [u0, u1)."""
                    py, px = scr[:, 0, u0:u1], scr[:, 1, u0:u1]
                    ty, tx = scr[:, 2, u0:u1], scr[:, 3, u0:u1]
                    y0, x0 = scr[:, 4, u0:u1], scr[:, 5, u0:u1]
                    t1 = scr[:, 6, u0:u1]
                    va, vb = scr[:, 7, u0:u1], scr[:, 8, u0:u1]
                    w4r = w4g[:, u0:u1]
                    qfr = qf_sb[:, u0:u1]

                    off_r = offs_g.rearrange("p n (k two) -> p n k two", two=2)
                    dy = off_r[:, u0:u1, :, 0]
                    dx = off_r[:, u0:u1, :, 1]
                    ybg = yb_sb[:, gg * GU + u0 : gg * GU + u1]
                    xbg = xb_sb[:, gg * GU + u0 : gg * GU + u1]
                    nc.vector.tensor_tensor(py, dy, ybg, Alu.add)
                    nc.vector.tensor_tensor(px, dx, xbg, Alu.add)
                    for (pp, tt, zz) in ((py, ty, y0), (px, tx, x0)):
                        nc.vector.tensor_scalar(
                            out=t1, in0=pp, scalar1=0.49999997, scalar2=MAGIC,
                            op0=Alu.subtract, op1=Alu.add,
                        )
                        nc.vector.tensor_scalar(
                            out=zz, in0=t1, scalar1=MAGIC, scalar2=None,
                            op0=Alu.subtract,
                        )
                        nc.vector.tensor_tensor(tt, pp, zz, Alu.subtract)

                    # corner weights -> w4[p, u, k, j]; j = (ycorner, xcorner)
                    # va = vy0 = (0<=y0<=95)*(1-ty); vb = vy1 = (-1<=y0<=94)*ty
                    nc.vector.tensor_scalar(out=t1, in0=y0, scalar1=0.0, scalar2=None, op0=Alu.is_ge)
                    nc.vector.tensor_scalar(out=va, in0=y0, scalar1=95.0, scalar2=None, op0=Alu.is_le)
                    nc.vector.tensor_tensor(va, va, t1, Alu.mult)
                    nc.vector.tensor_scalar(out=t1, in0=ty, scalar1=-1.0, scalar2=1.0, op0=Alu.mult, op1=Alu.add)
                    nc.vector.tensor_tensor(va, va, t1, Alu.mult)
                    nc.vector.tensor_scalar(out=t1, in0=y0, scalar1=-1.0, scalar2=None, op0=Alu.is_ge)
                    nc.vector.tensor_scalar(out=vb, in0=y0, scalar1=94.0, scalar2=None, op0=Alu.is_le)
                    nc.vector.tensor_tensor(vb, vb, t1, Alu.mult)
                    nc.vector.tensor_tensor(vb, vb, ty, Alu.mult)
                    # t1 = wx0, ty(reused) = wx1
                    wx0, wx1 = scr[:, 6, u0:u1], scr[:, 2, u0:u1]
                    nc.vector.tensor_scalar(out=wx0, in0=x0, scalar1=0.0, scalar2=None, op0=Alu.is_ge)
                    nc.vector.tensor_scalar(out=py, in0=x0, scalar1=95.0, scalar2=None, op0=Alu.is_le)
                    nc.vector.tensor_tensor(wx0, wx0, py, Alu.mult)
                    nc.vector.tensor_scalar(out=py, in0=tx, scalar1=-1.0, scalar2=1.0, op0=Alu.mult, op1=Alu.add)
                    nc.vector.tensor_tensor(wx0, wx0, py, Alu.mult)
                    nc.vector.tensor_scalar(out=wx1, in0=x0, scalar1=-1.0, scalar2=None, op0=Alu.is_ge)
                    nc.vector.tensor_scalar(out=py, in0=x0, scalar1=94.0, scalar2=None, op0=Alu.is_le)
                    nc.vector.tensor_tensor(wx1, wx1, py, Alu.mult)
                    nc.vector.tensor_tensor(wx1, wx1, tx, Alu.mult)

                    nc.vector.tensor_tensor(w4r[:, :, :, 0], va, wx0, Alu.mult)
                    nc.vector.tensor_tensor(w4r[:, :, :, 1], va, wx1, Alu.mult)
                    nc.vector.tensor_tensor(w4r[:, :, :, 2], vb, wx0, Alu.mult)
                    nc.vector.tensor_tensor(w4r[:, :, :, 3], vb, wx1, Alu.mult)

                    # gather index: clamp(y0,-1,95)*96 + clamp(x0,-1,95) + 97
                    nc.vector.tensor_scalar(out=va, in0=y0, scalar1=-1.0, scalar2=95.0, op0=Alu.max, op1=Alu.min)
                    nc.vector.tensor_scalar(out=vb, in0=x0, scalar1=-1.0, scalar2=95.0, op0=Alu.max, op1=Alu.min)
                    nc.vector.tensor_scalar(out=vb, in0=vb, scalar1=97.0, scalar2=None, op0=Alu.add)
                    nc.vector.scalar_tensor_tensor(
                        out=qfr, in0=va, scalar=96.0, in1=vb,
                        op0=Alu.mult, op1=Alu.add,
                    )

                def emit_wrap_range(u0, u1):
                    """int casts + SWDGE idx wrap for units [u0, u1).
                    Wrapped layout: idx j = k*128 + pp -> xwg[pp%16, u, k, pp//16]
                    via a coarse partition fold (8 contiguous DMAs) into
                    qi_t[g, s, u, k] + one DVE shuffle (direct strided-2-byte
                    wrap DMAs measured 2-5.6us each)."""
                    n = u1 - u0
                    if tap8_indirect:
                        nc.scalar.copy(qi32_8[:, u0:u1], qf_sb[:, u0:u1, 8])
                    nc.scalar.copy(qi_sb[:, u0:u1], qf_sb[:, u0:u1])
                    qi_t = qp.tile([16, 8, n, KK], dt.int16, tag=f"qit{n}",
                                   name=f"qi_t{n}")
                    for s in range(8):
                        eng = nc.sync if s % 2 == 0 else nc.scalar
                        eng.dma_start(
                            out=qi_t[:, s],
                            in_=qi_sb[s * 16 : (s + 1) * 16, u0:u1],
                        )
                    nc.vector.tensor_copy(
                        xwg[0:16, u0:u1], qi_t.rearrange("p s u k -> p u k s")
                    )
                    # replication tree 16 -> 128
                    nc.sync.dma_start(out=xwg[16:32, u0:u1], in_=xwg[0:16, u0:u1])
                    nc.sync.dma_start(out=xwg[32:64, u0:u1], in_=xwg[0:32, u0:u1])
                    nc.sync.dma_start(out=xwg[64:128, u0:u1], in_=xwg[0:64, u0:u1])

                if pipelined:
                    for bl in range(3):
                        emit_A_block(bl)
                        emit_B_range(bl * 4, bl * 4 + 4)
                        emit_wrap_range(bl * 4, bl * 4 + 4)
                else:
                    for bl in range(3):
                        emit_A_block(bl)
                    emit_B_range(0, GU)
                    emit_wrap_range(0, GU)
                return w4g, xwg, None, qi32_8

            def stage_CF(gg, w4g, xwg, xw8g, qi32_8):
                """Gather + combine + transpose + output matmuls, group gg."""
                qi32 = xwg if indirect else None
                for bl in range(3):
                    Bb = gg * 3 + bl
                    pms = [
                        ps_f.tile([128, 512], dt.float32, tag="pmf", name=f"pm{_oh}")
                        for _oh in range(2)
                    ]
                    gt8 = None
                    if tap8_batch and not indirect and not tap8_indirect:
                        # tap-8 for all 4 units of this block: one gather
                        gt8 = g_pool.tile([128, 4, 1024], dt.bfloat16, tag="g8")
                        nc.gpsimd.dma_gather(
                            out_ap=gt8[:],
                            in_ap=xt4_d[:],
                            idxs_ap=xw8g[:, bl].rearrange("p u g -> p (u g)"),
                            num_idxs=512,
                            num_idxs_reg=reg512,
                            elem_size=1024,
                            queue_num=0,
                        )
                    val_u = []
                    for qc in range(4):
                        ul = bl * 4 + qc
                        if indirect:
                            # HW-DGE indirect row gather, one per tap:
                            # gt[p, k, :] = xt4[qi32[p, ul, k], :]
                            gt = g_pool.tile([128, KK, 1024], dt.bfloat16, tag="g")
                            for k in range(KK):
                                nc.gpsimd.indirect_dma_start(
                                    out=gt[:, k],
                                    out_offset=None,
                                    in_=xt4_d[:],
                                    in_offset=bass.IndirectOffsetOnAxis(
                                        ap=qi32[:, ul, k : k + 1], axis=0
                                    ),
                                )
                        else:
                            # taps 0-7 (Q7 idx scratch caps num_idxs at 1024)
                            nt = 8 if (tap8_batch and not tap8_indirect) else KK
                            gt = g_pool.tile([128, nt, 1024], dt.bfloat16, tag="g")
                            nc.gpsimd.dma_gather(
                                out_ap=gt[:, 0:8],
                                in_ap=xt4_d[:],
                                idxs_ap=xwg[:, ul, 0:8].rearrange("p k g -> p (k g)"),
                                num_idxs=1024,
                                num_idxs_reg=reg1024,
                                elem_size=1024,
                                queue_num=0,
                            )
                            if tap8_indirect:
                                # tap 8 via HW-DGE indirect into the same tile
                                nc.gpsimd.indirect_dma_start(
                                    out=gt[:, 8],
                                    out_offset=None,
                                    in_=xt4_d[:],
                                    in_offset=bass.IndirectOffsetOnAxis(
                                        ap=qi32_8[:, ul : ul + 1],
                                        axis=0,
                                    ),
                                )
                            elif not tap8_batch:
                                nc.gpsimd.dma_gather(
                                    out_ap=gt[:, 8:9],
                                    in_ap=xt4_d[:],
                                    idxs_ap=xw8g[:, bl, qc],
                                    num_idxs=128,
                                    num_idxs_reg=reg128,
                                    elem_size=1024,
                                    queue_num=0,
                                )
                        # ACT: expand corner weights to c128 granularity
                        wexpc = wex_pool.tile([128, KK, 4, 128], dt.bfloat16, tag="wex")
                        nc.scalar.copy(
                            wexpc[:],
                            w4g[:, ul].unsqueeze(3).broadcast_to((128, KK, 4, 128)),
                        )
                        # DVE: packed-2x multiply (in place into gt) + folds
                        if tap8_batch and not indirect and not tap8_indirect:
                            mt = m_pool.tile([128, KK, 4, 2, 128], dt.bfloat16, tag="m")
                            for chh in range(2):
                                nc.vector.tensor_tensor(
                                    mt[:, 0:8, :, chh],
                                    gt.rearrange(
                                        "p k (j ch c) -> p k j ch c", j=4, ch=2
                                    )[:, :, :, chh],
                                    wexpc[:, 0:8],
                                    Alu.mult,
                                )
                                nc.vector.tensor_tensor(
                                    mt[:, 8:9, :, chh],
                                    gt8.rearrange(
                                        "p u (j ch c) -> p u j ch c", j=4, ch=2
                                    )[:, qc : qc + 1, :, chh],
                                    wexpc[:, 8:9],
                                    Alu.mult,
                                )
                            m = mt[:]
                        else:
                            mt = m_pool.tile([128, KK, 4, 2, 128], dt.bfloat16, tag="m")
                            for chh in range(2):
                                nc.vector.tensor_tensor(
                                    mt[:, :, :, chh],
                                    gt.rearrange(
                                        "p k (j ch c) -> p k j ch c", j=4, ch=2
                                    )[:, :, :, chh],
                                    wexpc[:],
                                    Alu.mult,
                                )
                            m = mt[:]
                        s = m_pool.tile([128, KK, 2, 2, 128], dt.bfloat16, tag="s")
                        nc.vector.tensor_tensor(
                            s[:], m[:, :, 0:2], m[:, :, 2:4], Alu.add
                        )
                        vu = val_pool.tile([128, CT * 128], dt.bfloat16, tag="val")
                        nc.vector.tensor_tensor(
                            vu.rearrange("p (k ch c) -> p k ch c", k=KK, ch=2),
                            s[:, :, 0],
                            s[:, :, 1],
                            Alu.add,
                        )
                        val_u.append(vu)
                    # PE transposes (2 ct / PSUM bank), software-pipelined
                    # with the accumulating output matmuls.
                    vm_tiles = [None] * (CT // 2)

                    def emit_E(ct2):
                        ptr = ps_e.tile([128, 2, 512], dt.bfloat16, tag="pe")
                        for half in range(2):
                            ct_i = ct2 * 2 + half
                            for qc in range(4):
                                nc.tensor.transpose(
                                    ptr[:, half, qc * 128 : (qc + 1) * 128],
                                    val_u[qc][:, ct_i * 128 : (ct_i + 1) * 128],
                                    identb_sb[:],
                                )
                        vmt = vm_pool.tile([128, 2, 512], dt.bfloat16, tag="vm")
                        nc.scalar.copy(vmt[:], ptr[:])
                        vm_tiles[ct2] = vmt

                    def emit_F(ct2):
                        vmt = vm_tiles[ct2]
                        for half in range(2):
                            ct_i = ct2 * 2 + half
                            for oh in range(2):
                                nc.tensor.matmul(
                                    pms[oh][:],
                                    lhsT=dw_sb[:, ct_i, oh * 128 : (oh + 1) * 128],
                                    rhs=vmt[:, half],
                                    start=(ct_i == 0),
                                    stop=(ct_i == CT - 1),
                                )

                    emit_E(0)
                    for ct2 in range(1, CT // 2):
                        emit_E(ct2)
                        emit_F(ct2 - 1)
                    emit_F(CT // 2 - 1)

                    for oh in range(2):
                        ob_t = outp.tile([128, 512], dt.float32, tag="ot")
                        nc.scalar.activation(
                            out=ob_t[:], in_=pms[oh][:],
                            func=Act.Identity, bias=db_sb[:, oh : oh + 1], scale=1.0,
                        )
                        nc.sync.dma_start(
                            out=out_d[oh, :, Bb * 512 : (Bb + 1) * 512], in_=ob_t[:]
                        )

            for _rep in range(repeat):
                xpg_a = load_xpg(0)
                xpg_b = load_xpg(1)
                wx0g = stage_AB(0, xpg_a, pipelined=(pipe0 and _rep == 0))
                xpg_c = load_xpg(2)
                wx1g = stage_AB(1, xpg_b)
                stage_CF(0, *wx0g)
                wx2g = stage_AB(2, xpg_c)
                stage_CF(1, *wx1g)
                stage_CF(2, *wx2g)

    # Post-scheduling queue assignment: Tile pins DMASW sem lane
    # (final_position % 8, over ALL Pool DMA insts) to whatever SWDGE
    # queue first updates it, so the queue must be a pure function of
    # the lane.  The scheduler may reorder Pool DMAs relative to
    # emission, so assign queues here from the final instruction order.
    from concourse.tile_scheduler import DMAInst

    lane = 0
    for f in nc.m.functions:
        for bb in f.blocks:
            for inst in bb.instructions:
                if inst.engine == mybir.EngineType.Pool and isinstance(
                    inst, DMAInst
                ):
                    if type(inst).__name__ == "InstDMAGatherAnt":
                        inst.queue_num = (lane % 8) % n_queues
                    lane += 1

    nc.compile()
    return nc


# ------------------------------------------------------------ main entry
_NC_CACHE = {}


def _get_nc():
    if "nc" not in _NC_CACHE:
        _NC_CACHE["nc"] = build_nc()
    return _NC_CACHE["nc"]


def _assemble(results):
    out = np.empty((B, O, H, W), dtype=np.float32)
    for core in range(8):
        b, half = core // 2, core % 2
        o = np.asarray(results[core]["out"]).reshape(O, ROWS, W)
        out[b, :, half * ROWS : (half + 1) * ROWS, :] = o
    return out


def kernel(x, offset_w, offset_b, deform_w, deform_b, **_ignored):
    from concourse.bass_utils import run_bass_kernel_spmd

    x = np.asarray(x, dtype=np.float32)
    offset_w = np.asarray(offset_w, dtype=np.float32)
    offset_b = np.asarray(offset_b, dtype=np.float32)
    deform_w = np.asarray(deform_w, dtype=np.float32)
    deform_b = np.asarray(deform_b, dtype=np.float32)

    nc = _get_nc()
    in_maps = [
        _prep_core_inputs(x, offset_w, offset_b, deform_w, deform_b, core)
        for core in range(8)
    ]
    res = run_bass_kernel_spmd(nc, in_maps, core_ids=list(range(8)))
    return _assemble([res.results[i] for i in range(8)])
